# revision 2
# baseline (speedup 1.0000x reference)
"""CharNGramVectorizer Trainium2 kernel — exact n-gram COUNT histogram.

out[b, v] = number of occurrences of n-gram v in row b (matches the
reference `.at[rows, idx].max(values)` as lowered on this stack, which
accumulates the all-ones payload).

Method (per core, 512 rows, data-parallel over 8 cores):
  codes (bf16, rows-major):   g2 = 5*t + t(+1),  g3 = 5*g2 + t(+2)
  PE-transpose code arrays to window-major [window, row] tiles.
  One-hot streams built on DVE: broadcast-copy code -> G_rep, then
  tensor_tensor is_equal against a constant iota pattern.
  PE matmul per (row, window-chunk), accumulating in PSUM over chunks:
    lhsT S = [onehot(g2) | onehot(t)]            [k=128, m=32]
    rhs  R = [oh(g3,+2) | oh(g2,+2) | oh(g2,+1)] [k=128, n=178]
    out[0:25, 0:125]   += count5 contributions  (P2' x P3(+2))
    out[0:25, 126:151] += count4                (P2' x P2(+2))
    out[26:31, 152:177]+= count3                (E   x P2(+1))
  PSUM (f32, exact) -> DRAM via strided DMA into [rows, 3875] layout.
Invalid/padding windows carry code 999 -> never match any iota value.
"""

import numpy as np

import concourse.bacc as bacc
import concourse.mybir as mybir
import concourse.tile as tile
from concourse.bass_utils import run_bass_kernel_spmd

N_CORES = 8
B, S = 4096, 512
VOCAB = 3875
RPC = B // N_CORES          # rows per core: 512
P = 128                     # rows per row-tile
NT = RPC // P               # 4 row-tiles
NC_CH = 4                   # window chunks of 128
RB = 32                     # rows per X-block / PSUM round
PAD = 999.0                 # code pad (positive bf16 pattern)
NOMATCH = -5.0              # iota pad value, never equals any code

SW = 32                     # stationary width: 25 g2 + 1 pad + 5 t + 1 pad
RW = 178                    # rhs width: 126 g3 + 26 g2 + 26 g2

bf16 = mybir.dt.bfloat16
f32 = mybir.dt.float32
i32 = mybir.dt.int32

mul = mybir.AluOpType.mult
add = mybir.AluOpType.add
is_eq = mybir.AluOpType.is_equal


def _iota_pattern():
    s = np.full(SW, NOMATCH, np.float32)
    s[0:25] = np.arange(25)
    s[26:31] = np.arange(5)
    r = np.full(RW, NOMATCH, np.float32)
    r[0:125] = np.arange(125)
    r[126:151] = np.arange(25)
    r[152:177] = np.arange(25)
    return s, r


def build():
    nc = bacc.Bacc("TRN2", target_bir_lowering=False, debug=False,
                   num_devices=N_CORES)
    tok_d = nc.dram_tensor("tokens", [RPC, S], i32, kind="ExternalInput")
    out_d = nc.dram_tensor("out", [RPC, VOCAB], f32, kind="ExternalOutput")

    s_pat, r_pat = _iota_pattern()
    import ml_dtypes
    ident_np = np.eye(128, dtype=ml_dtypes.bfloat16)
    iota_s_np = np.tile(np.tile(s_pat, RB)[None, :], (128, 1)).astype(ml_dtypes.bfloat16)
    iota_r_np = np.tile(np.tile(r_pat, RB)[None, :], (128, 1)).astype(ml_dtypes.bfloat16)
    ident_d = nc.inline_tensor(ident_np, "ident")
    iota_s_d = nc.inline_tensor(iota_s_np, "iota_s")
    iota_r_d = nc.inline_tensor(iota_r_np, "iota_r")

    with tile.TileContext(nc) as tc:
        with (
            tc.tile_pool(name="const", bufs=1) as cpool,
            tc.tile_pool(name="codes", bufs=2) as kpool,
            tc.tile_pool(name="xs", bufs=2) as xpool,
            tc.tile_pool(name="acc", bufs=8, space="PSUM") as apsum,
        ):
            ident = cpool.tile([128, 128], bf16)
            iota_s = cpool.tile([128, RB * SW], bf16)
            iota_r = cpool.tile([128, RB * RW], bf16)
            nc.sync.dma_start(out=ident[:], in_=ident_d[:])
            nc.sync.dma_start(out=iota_s[:], in_=iota_s_d[:])
            nc.sync.dma_start(out=iota_r[:], in_=iota_r_d[:])

            for rt in range(NT):
                tok32 = kpool.tile([P, S], i32, tag="tok32")
                nc.sync.dma_start(out=tok32[:], in_=tok_d[rt * P:(rt + 1) * P, :])

                t16 = kpool.tile([P, 514], bf16, tag="t16")
                g2 = kpool.tile([P, 514], bf16, tag="g2")
                g3 = kpool.tile([P, 514], bf16, tag="g3")
                nc.vector.tensor_copy(t16[:, 0:512], tok32[:])
                nc.vector.memset(t16[:, 512:514], PAD)
                nc.vector.tensor_scalar(g2[:, 0:511], t16[:, 0:511], 5.0, None, mul)
                nc.vector.tensor_tensor(g2[:, 0:511], g2[:, 0:511], t16[:, 1:512], add)
                nc.vector.memset(g2[:, 511:514], PAD)
                nc.vector.tensor_scalar(g3[:, 0:510], g2[:, 0:510], 5.0, None, mul)
                nc.vector.tensor_tensor(g3[:, 0:510], g3[:, 0:510], t16[:, 2:512], add)
                nc.vector.memset(g3[:, 510:514], PAD)

                # window-major code tiles: [128 windows, 4 chunks x 128 rows]
                srcs = [t16[:, 0:512], g2[:, 0:512], g2[:, 1:513],
                        g2[:, 2:514], g3[:, 2:514]]
                codesT = [kpool.tile([P, 4 * P], bf16, tag=f"cT{i}", name=f"cT{i}")
                          for i in range(5)]
                for i, src in enumerate(srcs):
                    for c in range(NC_CH):
                        pt = apsum.tile([128, 128], bf16, tag="acc", name="tp")
                        nc.tensor.transpose(pt[:], src[:, c * 128:(c + 1) * 128],
                                            ident[:])
                        nc.scalar.activation(
                            codesT[i][:, c * 128:(c + 1) * 128], pt[:],
                            mybir.ActivationFunctionType.Copy)
                cT_t, cT_g2s0, cT_g2s1, cT_g2s2, cT_g3s2 = codesT

                # int32 "pair" views: each value = bf16 code bits x 65537,
                # i.e. the code bit-pattern duplicated in both 16-bit halves
                i16 = mybir.dt.int16
                pairs = []
                for i in range(5):
                    w = kpool.tile([P, 4 * P], i32, tag=f"pw{i}", name=f"pw{i}")
                    wh = w[:].bitcast(i16).rearrange("p (v two) -> p v two", two=2)
                    cs = codesT[i][:].bitcast(i16)
                    nc.vector.tensor_copy(wh[:, :, 0:1], cs.unsqueeze(2))
                    nc.vector.tensor_copy(wh[:, :, 1:2], cs.unsqueeze(2))
                    pairs.append(w)
                p_t, p_g2s0, p_g2s1, p_g2s2, p_g3s2 = pairs

                stage = kpool.tile([P, (P // RB) * (RB // 4) * 178], f32, tag="stage")
                for blk in range(P // RB):        # 16 blocks of 8 rows
                    banks = [apsum.tile([128, 512], f32, tag="acc",
                                        name=f"bank{q}") for q in range(RB // 4)]

                    for c in range(NC_CH):
                        xs = xpool.tile([P, RB * SW], bf16, tag="xs")
                        xr = xpool.tile([P, RB * RW], bf16, tag="xr")
                        x3s = xs[:].rearrange("p (r v) -> p r v", r=RB)
                        x3r = xr[:].rearrange("p (r v) -> p r v", r=RB)

                        def codecol(ct, width):
                            o = c * 128 + blk * RB
                            return ct[:, o:o + RB].unsqueeze(2).broadcast_to(
                                [P, RB, width])

                        xsi = xs[:].bitcast(i32).rearrange("p (r v) -> p r v", r=RB)
                        xri = xr[:].bitcast(i32).rearrange("p (r v) -> p r v", r=RB)
                        # build S: [0:26) g2s0, [26:32) t  (i32 pair writes)
                        nc.vector.tensor_copy(xsi[:, :, 0:13], codecol(p_g2s0, 13))
                        nc.vector.tensor_copy(xsi[:, :, 13:16], codecol(p_t, 3))
                        nc.vector.tensor_tensor(xs[:], xs[:], iota_s[:], is_eq)
                        # build R: [0:126) g3s2, [126:152) g2s2, [152:178) g2s1
                        nc.vector.tensor_copy(xri[:, :, 0:63], codecol(p_g3s2, 63))
                        nc.vector.tensor_copy(xri[:, :, 63:76], codecol(p_g2s2, 13))
                        nc.vector.tensor_copy(xri[:, :, 76:89], codecol(p_g2s1, 13))
                        nc.vector.tensor_tensor(xr[:], xr[:], iota_r[:], is_eq)

                        for l in range(RB):
                            j, u = l % 4, l // 4
                            nc.tensor.matmul(
                                banks[u][32 * j:32 * j + 32, 0:RW],
                                x3s[:, l, 0:SW],
                                x3r[:, l, 0:RW],
                                start=(c == 0), stop=(c == NC_CH - 1),
                                tile_position=(0, 32 * j))

                    for u in range(RB // 4):
                        o = (blk * (RB // 4) + u) * 178
                        nc.scalar.activation(
                            stage[:, o:o + 178], banks[u][:, 0:178],
                            mybir.ActivationFunctionType.Copy)

                # DMAs: rows r = rt*128 + blk*8 + u*4 + j
                ov = out_d[rt * P:(rt + 1) * P, :].rearrange(
                    "(blk s) v -> s blk v", s=RB)
                st3 = stage[:].rearrange("q (blk w) -> q blk w", blk=P // RB)
                for u in range(RB // 4):
                    for j in range(4):
                        row = u * 4 + j
                        nc.sync.dma_start(
                            out=ov[row, :, 750:3875].rearrange(
                                "blk (p w) -> p blk w", p=25),
                            in_=st3[32 * j:32 * j + 25, :,
                                    u * 178:u * 178 + 125])
                        nc.sync.dma_start(
                            out=ov[row, :, 125:750].rearrange(
                                "blk (p w) -> p blk w", p=25),
                            in_=st3[32 * j:32 * j + 25, :,
                                    u * 178 + 126:u * 178 + 151])
                        nc.sync.dma_start(
                            out=ov[row, :, 0:125].rearrange(
                                "blk (p w) -> p blk w", p=5),
                            in_=st3[32 * j + 26:32 * j + 31, :,
                                    u * 178 + 152:u * 178 + 177])

    nc.compile()
    return nc


_NC = None


def _get_nc():
    global _NC
    if _NC is None:
        _NC = build()
    return _NC


def run_sharded(tokens: np.ndarray, trace: bool = False):
    nc = _get_nc()
    tokens = np.ascontiguousarray(tokens, dtype=np.int32)
    in_maps = [{"tokens": tokens[c * RPC:(c + 1) * RPC]} for c in range(N_CORES)]
    res = run_bass_kernel_spmd(nc, in_maps, core_ids=list(range(N_CORES)),
                               trace=trace)
    out = np.concatenate([res.results[c]["out"] for c in range(N_CORES)], axis=0)
    return out, res


def kernel(tokens: np.ndarray, values: np.ndarray) -> np.ndarray:
    out, _ = run_sharded(tokens)
    return out


# revision 3
# speedup vs baseline: 1.0772x; 1.0772x over previous
"""CharNGramVectorizer Trainium2 kernel — exact n-gram COUNT histogram.

out[b, v] = number of occurrences of n-gram v in row b (matches the
reference `.at[rows, idx].max(values)` as lowered on this stack, which
accumulates the all-ones payload).

Method (per core, 512 rows, data-parallel over 8 cores):
  codes (bf16, rows-major):   g2 = 5*t + t(+1),  g3 = 5*g2 + t(+2)
  PE-transpose code arrays to window-major [window, row] tiles.
  One-hot streams built on DVE: broadcast-copy code -> G_rep, then
  tensor_tensor is_equal against a constant iota pattern.
  PE matmul per (row, window-chunk), accumulating in PSUM over chunks:
    lhsT S = [onehot(g2) | onehot(t)]            [k=128, m=32]
    rhs  R = [oh(g3,+2) | oh(g2,+2) | oh(g2,+1)] [k=128, n=178]
    out[0:25, 0:125]   += count5 contributions  (P2' x P3(+2))
    out[0:25, 126:151] += count4                (P2' x P2(+2))
    out[26:31, 152:177]+= count3                (E   x P2(+1))
  PSUM (f32, exact) -> DRAM via strided DMA into [rows, 3875] layout.
Invalid/padding windows carry code 999 -> never match any iota value.
"""

import numpy as np

import concourse.bacc as bacc
import concourse.mybir as mybir
import concourse.tile as tile
from concourse.bass_utils import run_bass_kernel_spmd

N_CORES = 8
B, S = 4096, 512
VOCAB = 3875
RPC = B // N_CORES          # rows per core: 512
P = 128                     # rows per row-tile
NT = RPC // P               # 4 row-tiles
NC_CH = 4                   # window chunks of 128
RB = 32                     # rows per X-block / PSUM round
PAD = 999.0                 # code pad (positive bf16 pattern)
NOMATCH = -5.0              # iota pad value, never equals any code

SW = 32                     # stationary width: 25 g2 + 1 pad + 5 t + 1 pad
RW = 178                    # rhs width: 126 g3 + 26 g2 + 26 g2

bf16 = mybir.dt.bfloat16
f32 = mybir.dt.float32
i32 = mybir.dt.int32

mul = mybir.AluOpType.mult
add = mybir.AluOpType.add
is_eq = mybir.AluOpType.is_equal


def _iota_pattern():
    s = np.full(SW, NOMATCH, np.float32)
    s[0:25] = np.arange(25)
    s[26:31] = np.arange(5)
    r = np.full(RW, NOMATCH, np.float32)
    r[0:125] = np.arange(125)
    r[126:151] = np.arange(25)
    r[152:177] = np.arange(25)
    return s, r


def build():
    nc = bacc.Bacc("TRN2", target_bir_lowering=False, debug=False,
                   num_devices=N_CORES)
    tok_d = nc.dram_tensor("tokens", [RPC, S], i32, kind="ExternalInput")
    out_d = nc.dram_tensor("out", [RPC, VOCAB], f32, kind="ExternalOutput")

    s_pat, r_pat = _iota_pattern()
    import ml_dtypes
    ident_np = np.eye(128, dtype=ml_dtypes.bfloat16)
    iota_s_np = np.tile(np.tile(s_pat, RB)[None, :], (128, 1)).astype(ml_dtypes.bfloat16)
    iota_r_np = np.tile(np.tile(r_pat, RB)[None, :], (128, 1)).astype(ml_dtypes.bfloat16)
    ident_d = nc.inline_tensor(ident_np, "ident")
    iota_s_d = nc.inline_tensor(iota_s_np, "iota_s")
    iota_r_d = nc.inline_tensor(iota_r_np, "iota_r")

    with tile.TileContext(nc) as tc:
        with (
            tc.tile_pool(name="const", bufs=1) as cpool,
            tc.tile_pool(name="codes", bufs=2) as kpool,
            tc.tile_pool(name="xs", bufs=4) as xpool,
            tc.tile_pool(name="acc", bufs=8, space="PSUM") as apsum,
        ):
            ident = cpool.tile([128, 128], bf16)
            iota_s = cpool.tile([128, RB * SW], bf16)
            iota_r = cpool.tile([128, RB * RW], bf16)
            nc.sync.dma_start(out=ident[:], in_=ident_d[:])
            nc.sync.dma_start(out=iota_s[:], in_=iota_s_d[:])
            nc.sync.dma_start(out=iota_r[:], in_=iota_r_d[:])

            for rt in range(NT):
                tok32 = kpool.tile([P, S], i32, tag="tok32")
                nc.sync.dma_start(out=tok32[:], in_=tok_d[rt * P:(rt + 1) * P, :])

                t16 = kpool.tile([P, 514], bf16, tag="t16")
                g2 = kpool.tile([P, 514], bf16, tag="g2")
                g3 = kpool.tile([P, 514], bf16, tag="g3")
                nc.vector.tensor_copy(t16[:, 0:512], tok32[:])
                nc.vector.memset(t16[:, 512:514], PAD)
                nc.vector.tensor_scalar(g2[:, 0:511], t16[:, 0:511], 5.0, None, mul)
                nc.vector.tensor_tensor(g2[:, 0:511], g2[:, 0:511], t16[:, 1:512], add)
                nc.vector.memset(g2[:, 511:514], PAD)
                nc.vector.tensor_scalar(g3[:, 0:510], g2[:, 0:510], 5.0, None, mul)
                nc.vector.tensor_tensor(g3[:, 0:510], g3[:, 0:510], t16[:, 2:512], add)
                nc.vector.memset(g3[:, 510:514], PAD)

                # window-major code tiles: [128 windows, 4 chunks x 128 rows]
                srcs = [t16[:, 0:512], g2[:, 0:512], g2[:, 1:513],
                        g2[:, 2:514], g3[:, 2:514]]
                codesT = [kpool.tile([P, 4 * P], bf16, tag=f"cT{i}", name=f"cT{i}")
                          for i in range(5)]
                for i, src in enumerate(srcs):
                    for c in range(NC_CH):
                        pt = apsum.tile([128, 128], bf16, tag="acc", name="tp")
                        nc.tensor.transpose(pt[:], src[:, c * 128:(c + 1) * 128],
                                            ident[:])
                        nc.scalar.activation(
                            codesT[i][:, c * 128:(c + 1) * 128], pt[:],
                            mybir.ActivationFunctionType.Copy)
                cT_t, cT_g2s0, cT_g2s1, cT_g2s2, cT_g3s2 = codesT

                # int32 "pair" views: each value = bf16 code bits x 65537,
                # i.e. the code bit-pattern duplicated in both 16-bit halves
                i16 = mybir.dt.int16
                pairs = []
                for i in range(5):
                    w = kpool.tile([P, 4 * P], i32, tag=f"pw{i}", name=f"pw{i}")
                    wh = w[:].bitcast(i16).rearrange("p (v two) -> p v two", two=2)
                    cs = codesT[i][:].bitcast(i16)
                    nc.vector.tensor_copy(wh[:, :, 0:1], cs.unsqueeze(2))
                    nc.vector.tensor_copy(wh[:, :, 1:2], cs.unsqueeze(2))
                    pairs.append(w)
                p_t, p_g2s0, p_g2s1, p_g2s2, p_g3s2 = pairs

                stage = kpool.tile([P, (P // RB) * (RB // 4) * 178], f32, tag="stage")
                for blk in range(P // RB):        # 16 blocks of 8 rows
                    banks = [apsum.tile([128, 512], f32, tag="acc",
                                        name=f"bank{q}") for q in range(RB // 4)]

                    for c in range(NC_CH):
                        xs = xpool.tile([P, RB * SW], bf16, tag="xs")
                        xr = xpool.tile([P, RB * RW], bf16, tag="xr")
                        x3s = xs[:].rearrange("p (r v) -> p r v", r=RB)
                        x3r = xr[:].rearrange("p (r v) -> p r v", r=RB)

                        def codecol(ct, width):
                            o = c * 128 + blk * RB
                            return ct[:, o:o + RB].unsqueeze(2).broadcast_to(
                                [P, RB, width])

                        xsi = xs[:].bitcast(i32).rearrange("p (r v) -> p r v", r=RB)
                        xri = xr[:].bitcast(i32).rearrange("p (r v) -> p r v", r=RB)
                        # build S: [0:26) g2s0, [26:32) t  (i32 pair writes)
                        nc.vector.tensor_copy(xsi[:, :, 0:13], codecol(p_g2s0, 13))
                        nc.vector.tensor_copy(xsi[:, :, 13:16], codecol(p_t, 3))
                        nc.vector.tensor_tensor(xs[:], xs[:], iota_s[:], is_eq)
                        # build R: [0:126) g3s2, [126:152) g2s2, [152:178) g2s1
                        nc.vector.tensor_copy(xri[:, :, 0:63], codecol(p_g3s2, 63))
                        nc.vector.tensor_copy(xri[:, :, 63:76], codecol(p_g2s2, 13))
                        nc.vector.tensor_copy(xri[:, :, 76:89], codecol(p_g2s1, 13))
                        nc.vector.tensor_tensor(xr[:], xr[:], iota_r[:], is_eq)

                        for l in range(RB):
                            j, u = l % 4, l // 4
                            nc.tensor.matmul(
                                banks[u][32 * j:32 * j + 32, 0:RW],
                                x3s[:, l, 0:SW],
                                x3r[:, l, 0:RW],
                                start=(c == 0), stop=(c == NC_CH - 1),
                                tile_position=(0, 32 * j))

                    for u in range(RB // 4):
                        o = (blk * (RB // 4) + u) * 178
                        nc.scalar.activation(
                            stage[:, o:o + 178], banks[u][:, 0:178],
                            mybir.ActivationFunctionType.Copy)

                # DMAs: rows r = rt*128 + 4*(blk*8+u) + j == 4*bu + j
                ovj = out_d[rt * P:(rt + 1) * P, :].rearrange(
                    "(bu j) v -> j bu v", j=4)
                stm = stage[:].rearrange("q (bu w) -> q bu w", bu=32)
                for j in range(4):
                    nc.sync.dma_start(
                        out=ovj[j, :, 750:3875].rearrange(
                            "bu (p w) -> p bu w", p=25),
                        in_=stm[32 * j:32 * j + 25, :, 0:125])
                    nc.gpsimd.dma_start(
                        out=ovj[j, :, 125:750].rearrange(
                            "bu (p w) -> p bu w", p=25),
                        in_=stm[32 * j:32 * j + 25, :, 126:151])
                    nc.sync.dma_start(
                        out=ovj[j, :, 0:125].rearrange(
                            "bu (p w) -> p bu w", p=5),
                        in_=stm[32 * j + 26:32 * j + 31, :, 152:177])

    nc.compile()
    return nc


_NC = None


def _get_nc():
    global _NC
    if _NC is None:
        _NC = build()
    return _NC


def run_sharded(tokens: np.ndarray, trace: bool = False):
    nc = _get_nc()
    tokens = np.ascontiguousarray(tokens, dtype=np.int32)
    in_maps = [{"tokens": tokens[c * RPC:(c + 1) * RPC]} for c in range(N_CORES)]
    res = run_bass_kernel_spmd(nc, in_maps, core_ids=list(range(N_CORES)),
                               trace=trace)
    out = np.concatenate([res.results[c]["out"] for c in range(N_CORES)], axis=0)
    return out, res


def kernel(tokens: np.ndarray, values: np.ndarray) -> np.ndarray:
    out, _ = run_sharded(tokens)
    return out


# revision 4
# speedup vs baseline: 1.2243x; 1.1365x over previous
"""CharNGramVectorizer Trainium2 kernel — exact n-gram COUNT histogram.

out[b, v] = number of occurrences of n-gram v in row b (matches the
reference `.at[rows, idx].max(values)` as lowered on this stack, which
accumulates the all-ones payload).

Method (per core, 512 rows, data-parallel over 8 cores):
  codes (bf16, rows-major):   g2 = 5*t + t(+1),  g3 = 5*g2 + t(+2)
  PE-transpose code arrays to window-major [window, row] tiles.
  One-hot streams built on DVE: broadcast-copy code -> G_rep, then
  tensor_tensor is_equal against a constant iota pattern.
  PE matmul per (row, window-chunk), accumulating in PSUM over chunks:
    lhsT S = [onehot(g2) | onehot(t)]            [k=128, m=32]
    rhs  R = [oh(g3,+2) | oh(g2,+2) | oh(g2,+1)] [k=128, n=178]
    out[0:25, 0:125]   += count5 contributions  (P2' x P3(+2))
    out[0:25, 126:151] += count4                (P2' x P2(+2))
    out[26:31, 152:177]+= count3                (E   x P2(+1))
  PSUM (f32, exact) -> DRAM via strided DMA into [rows, 3875] layout.
Invalid/padding windows carry code 999 -> never match any iota value.
"""

import numpy as np

import concourse.bacc as bacc
import concourse.mybir as mybir
import concourse.tile as tile
from concourse.bass_utils import run_bass_kernel_spmd

N_CORES = 8
B, S = 4096, 512
VOCAB = 3875
RPC = B // N_CORES          # rows per core: 512
P = 128                     # rows per row-tile
NT = RPC // P               # 4 row-tiles
NC_CH = 4                   # window chunks of 128
RB = 32                     # rows per X-block / PSUM round
PAD = 999.0                 # code pad (positive bf16 pattern)
NOMATCH = -5.0              # iota pad value, never equals any code

SW = 32                     # stationary width: 25 g2 + 1 pad + 5 t + 1 pad
RW = 178                    # rhs width: 126 g3 + 26 g2 + 26 g2

bf16 = mybir.dt.bfloat16
f32 = mybir.dt.float32
i32 = mybir.dt.int32

mul = mybir.AluOpType.mult
add = mybir.AluOpType.add
is_eq = mybir.AluOpType.is_equal


def _iota_pattern():
    s = np.full(SW, NOMATCH, np.float32)
    s[0:25] = np.arange(25)
    s[26:31] = np.arange(5)
    r = np.full(RW, NOMATCH, np.float32)
    r[0:125] = np.arange(125)
    r[126:151] = np.arange(25)
    r[152:177] = np.arange(25)
    return np.concatenate([s, r])


def build():
    nc = bacc.Bacc("TRN2", target_bir_lowering=False, debug=False,
                   num_devices=N_CORES)
    tok_d = nc.dram_tensor("tokens", [RPC, S], i32, kind="ExternalInput")
    out_d = nc.dram_tensor("out", [RPC, VOCAB], f32, kind="ExternalOutput")

    sr_pat = _iota_pattern()
    import ml_dtypes
    ident_np = np.eye(128, dtype=ml_dtypes.bfloat16)
    iota_sr_np = np.tile(np.tile(sr_pat, RB)[None, :], (128, 1)).astype(ml_dtypes.bfloat16)
    ident_d = nc.inline_tensor(ident_np, "ident")
    iota_sr_d = nc.inline_tensor(iota_sr_np, "iota_sr")

    with tile.TileContext(nc) as tc:
        with (
            tc.tile_pool(name="const", bufs=1) as cpool,
            tc.tile_pool(name="codes", bufs=2) as kpool,
            tc.tile_pool(name="xs", bufs=4) as xpool,
            tc.tile_pool(name="acc", bufs=8, space="PSUM") as apsum,
        ):
            XW = SW + RW
            ident = cpool.tile([128, 128], bf16)
            iota_sr = cpool.tile([128, RB * (SW + RW)], bf16)
            nc.sync.dma_start(out=ident[:], in_=ident_d[:])
            nc.sync.dma_start(out=iota_sr[:], in_=iota_sr_d[:])

            for rt in range(NT):
                tok32 = kpool.tile([P, S], i32, tag="tok32")
                nc.sync.dma_start(out=tok32[:], in_=tok_d[rt * P:(rt + 1) * P, :])

                t16 = kpool.tile([P, 514], bf16, tag="t16")
                g2 = kpool.tile([P, 514], bf16, tag="g2")
                g3 = kpool.tile([P, 514], bf16, tag="g3")
                nc.vector.tensor_copy(t16[:, 0:512], tok32[:])
                nc.vector.memset(t16[:, 512:514], PAD)
                nc.vector.tensor_scalar(g2[:, 0:511], t16[:, 0:511], 5.0, None, mul)
                nc.vector.tensor_tensor(g2[:, 0:511], g2[:, 0:511], t16[:, 1:512], add)
                nc.vector.memset(g2[:, 511:514], PAD)
                nc.vector.tensor_scalar(g3[:, 0:510], g2[:, 0:510], 5.0, None, mul)
                nc.vector.tensor_tensor(g3[:, 0:510], g3[:, 0:510], t16[:, 2:512], add)
                nc.vector.memset(g3[:, 510:514], PAD)

                # window-major code tiles: [128 windows, 4 chunks x 128 rows]
                srcs = [t16[:, 0:512], g2[:, 0:512], g2[:, 1:513],
                        g2[:, 2:514], g3[:, 2:514]]
                codesT = [kpool.tile([P, 4 * P], bf16, tag=f"cT{i}", name=f"cT{i}")
                          for i in range(5)]
                for i, src in enumerate(srcs):
                    for c in range(NC_CH):
                        pt = apsum.tile([128, 128], bf16, tag="acc", name="tp")
                        nc.tensor.transpose(pt[:], src[:, c * 128:(c + 1) * 128],
                                            ident[:])
                        nc.scalar.activation(
                            codesT[i][:, c * 128:(c + 1) * 128], pt[:],
                            mybir.ActivationFunctionType.Copy)
                cT_t, cT_g2s0, cT_g2s1, cT_g2s2, cT_g3s2 = codesT

                # int32 "pair" views: each value = bf16 code bits x 65537,
                # i.e. the code bit-pattern duplicated in both 16-bit halves
                i16 = mybir.dt.int16
                pairs = []
                for i in range(5):
                    w = kpool.tile([P, 4 * P], i32, tag=f"pw{i}", name=f"pw{i}")
                    wh = w[:].bitcast(i16).rearrange("p (v two) -> p v two", two=2)
                    cs = codesT[i][:].bitcast(i16)
                    nc.vector.tensor_copy(wh[:, :, 0:1], cs.unsqueeze(2))
                    nc.vector.tensor_copy(wh[:, :, 1:2], cs.unsqueeze(2))
                    pairs.append(w)
                p_t, p_g2s0, p_g2s1, p_g2s2, p_g3s2 = pairs

                stage = kpool.tile([P, (P // RB) * (RB // 4) * 178], f32, tag="stage")
                for blk in range(P // RB):        # 16 blocks of 8 rows
                    banks = [apsum.tile([128, 512], f32, tag="acc",
                                        name=f"bank{q}") for q in range(RB // 4)]

                    for c in range(NC_CH):
                        x = xpool.tile([P, RB * (SW + RW)], bf16, tag="x")
                        x3 = x[:].rearrange("p (r v) -> p r v", r=RB)

                        def codecol(ct, width):
                            o = c * 128 + blk * RB
                            return ct[:, o:o + RB].unsqueeze(2).broadcast_to(
                                [P, RB, width])

                        xi = x[:].bitcast(i32).rearrange("p (r v) -> p r v", r=RB)
                        # S: [0:26) g2s0, [26:32) t; R: [32:158) g3s2,
                        # [158:184) g2s2, [184:210) g2s1   (i32 pair writes)
                        nc.vector.tensor_copy(xi[:, :, 0:13], codecol(p_g2s0, 13))
                        nc.vector.tensor_copy(xi[:, :, 13:16], codecol(p_t, 3))
                        o2 = c * 128 + blk * RB
                        nc.scalar.activation(
                            x3[:, :, SW:SW + 126],
                            cT_g3s2[:, o2:o2 + RB].unsqueeze(2).broadcast_to(
                                [P, RB, 126]),
                            mybir.ActivationFunctionType.Copy)
                        nc.vector.tensor_copy(xi[:, :, 79:92], codecol(p_g2s2, 13))
                        nc.vector.tensor_copy(xi[:, :, 92:105], codecol(p_g2s1, 13))
                        nc.vector.tensor_tensor(x[:], x[:], iota_sr[:], is_eq)

                        for l in range(RB):
                            j, u = l % 4, l // 4
                            nc.tensor.matmul(
                                banks[u][32 * j:32 * j + 32, 0:RW],
                                x3[:, l, 0:SW],
                                x3[:, l, SW:SW + RW],
                                start=(c == 0), stop=(c == NC_CH - 1),
                                tile_position=(0, 32 * j))

                    for u in range(RB // 4):
                        o = (blk * (RB // 4) + u) * 178
                        nc.scalar.activation(
                            stage[:, o:o + 178], banks[u][:, 0:178],
                            mybir.ActivationFunctionType.Copy)

                # DMAs: rows r = rt*128 + 4*(blk*8+u) + j == 4*bu + j
                ovj = out_d[rt * P:(rt + 1) * P, :].rearrange(
                    "(bu j) v -> j bu v", j=4)
                stm = stage[:].rearrange("q (bu w) -> q bu w", bu=32)
                for j in range(4):
                    nc.sync.dma_start(
                        out=ovj[j, :, 750:3875].rearrange(
                            "bu (p w) -> p bu w", p=25),
                        in_=stm[32 * j:32 * j + 25, :, 0:125])
                    nc.gpsimd.dma_start(
                        out=ovj[j, :, 125:750].rearrange(
                            "bu (p w) -> p bu w", p=25),
                        in_=stm[32 * j:32 * j + 25, :, 126:151])
                    nc.sync.dma_start(
                        out=ovj[j, :, 0:125].rearrange(
                            "bu (p w) -> p bu w", p=5),
                        in_=stm[32 * j + 26:32 * j + 31, :, 152:177])

    nc.compile()
    return nc


_NC = None


def _get_nc():
    global _NC
    if _NC is None:
        _NC = build()
    return _NC


def run_sharded(tokens: np.ndarray, trace: bool = False):
    nc = _get_nc()
    tokens = np.ascontiguousarray(tokens, dtype=np.int32)
    in_maps = [{"tokens": tokens[c * RPC:(c + 1) * RPC]} for c in range(N_CORES)]
    res = run_bass_kernel_spmd(nc, in_maps, core_ids=list(range(N_CORES)),
                               trace=trace)
    out = np.concatenate([res.results[c]["out"] for c in range(N_CORES)], axis=0)
    return out, res


def kernel(tokens: np.ndarray, values: np.ndarray) -> np.ndarray:
    out, _ = run_sharded(tokens)
    return out
